# revision 9
# baseline (speedup 1.0000x reference)
"""Trainium2 Bass kernel for nn_KnowledgeDifficulty (ragged active-column version).

Math (per batch b):
  logits = X[b] @ Wa + ba            (N, M)
  w      = softmax(logits, axis=N)   -- ba constant along N => cancels
  d      = sigmoid((sum_n e[n,m] * y[n]) / (sum_n e[n,m]) + bs)
    where e = exp(logits), y = X[b] @ Ws
  out    = d * (K > 0)

Ragged trick: only the ~50% columns with K>0 are needed.  The host gathers
the active columns of Wa per batch (scaled by log2(e): device works in
exp2 units), pads each batch to a fixed per-slot width, and scatters the
packed device output back to the full (B, M) array.

Device (per core, 8 batches = 8 "slots"):
  mm1 (bf16): logits2[n, c] = xt_chunk^T @ wsab (wsab = [Wa_act*log2e | Ws]),
       column W of each chunk region holds y for that chunk.
  exp: each PSUM tile's columns are split between ACT (spline exp,
       scale=ln2) and DVE (Schraudolph: i16 = round(z*128 + 16250.49)
       bitcast to bf16 ~= 2^z, +-3% rel) so both engines run on every
       tile concurrently.
  mm2 (bf16): per group of 4 slots, PE column-group packing: lhsT=[y|1]
       gives rows t=sum(e*y), s=sum(e) at partitions 32j/32j+1.
  epilogue: DVE 32x32 block-transpose puts t/s into columns; vectorized
       recip/mul/exp/add/recip gives d; one small DMA out, host unshuffles.

Batches are assigned to (core, slot) sorted by active count so each slot's
static width is tight; widths are baked into the NEFF at first call.
"""

import numpy as np

B, N, L, M = 64, 512, 128, 1024
NCORES = 8
BLOC = B // NCORES  # 8 slots per core
LOG2E = 1.4426950408889634
LN2 = 0.6931471805599453
SCHRAUD_B = 16256.0 - 5.51  # bf16 exp2 bitcast bias (round-to-nearest)
ACT_FRAC = 0.55  # fraction of each tile's exp columns done by ACT

_STATE = {}


def _pieces(lo, hi):
    # split [lo, hi) at multiples of 512 (PSUM bank = 512 fp32)
    out = []
    while lo < hi:
        nxt = min(hi, (lo // 512 + 1) * 512)
        out.append((lo, nxt))
        lo = nxt
    return out


def _build(Wlist, KB):
    import concourse.bacc as bacc
    import concourse.tile as tile
    import concourse.mybir as mybir

    f32 = mybir.dt.float32
    bf16 = mybir.dt.bfloat16
    i16 = mybir.dt.int16
    Exp = mybir.ActivationFunctionType.Exp
    MULT = mybir.AluOpType.mult
    ADD = mybir.AluOpType.add

    WMAX = Wlist[0]
    TW = 2 * WMAX + 2  # lg tile width (2 chunks + 2 y cols)

    nc = bacc.Bacc(
        "TRN2", target_bir_lowering=False, debug=False, num_devices=NCORES
    )
    wsab_d = [
        nc.dram_tensor(f"wsab{s}", (L, Wlist[s] + 1), bf16, kind="ExternalInput")
        for s in range(BLOC)
    ]
    xt_d = nc.dram_tensor("xt", (BLOC, L, N), bf16, kind="ExternalInput")
    bn_d = nc.dram_tensor("bn", (128, 1), f32, kind="ExternalInput")
    out_d = nc.dram_tensor("out", (128, 2, KB), f32, kind="ExternalOutput")

    with tile.TileContext(nc) as tc:
        with (
            tc.tile_pool(name="sb", bufs=1) as sb,
            tc.tile_pool(name="lgp", bufs=2, space="PSUM") as lgp,
            tc.tile_pool(name="o2p", bufs=1, space="PSUM") as o2p,
        ):
            bn_sb = sb.tile([128, 1], f32, name="bn_sb")
            # y2 interleaved: col 2k = y for (slot,chunk) k, col 2k+1 = 1.0
            y2 = sb.tile([128, 64], bf16, name="y2")
            y2v = y2.rearrange("q (k two) -> q k two", two=2)
            nc.vector.memset(y2v[:, :, 1], 1.0)

            xt_sb = sb.tile([L, BLOC, N], bf16, name="xt_sb")
            wsab_sb = []
            # slot 0 split finely over sync+gpsimd (scalar queue does the
            # ACT table load first); later slots rotate over all three
            for s in range(BLOC):
                w_sb = sb.tile(
                    [L, Wlist[s] + 1], bf16, tag=f"w{s}", name=f"w{s}"
                )
                wsab_sb.append(w_sb)
            qs = [nc.sync, nc.gpsimd, nc.scalar]
            for h, (plo, phi) in enumerate(((0, 43), (43, 86), (86, 128))):
                qs[h].dma_start(
                    wsab_sb[0][plo:phi, :], wsab_d[0][plo:phi, :]
                )
            nc.sync.dma_start(xt_sb[:, 0, 0:128], xt_d[0][:, 0:128])
            nc.gpsimd.dma_start(xt_sb[:, 0, 128:256], xt_d[0][:, 128:256])
            nc.scalar.dma_start(xt_sb[:, 0, 256:512], xt_d[0][:, 256:512])
            qi = 2
            for s in range(1, BLOC):
                qs[qi % 3].dma_start(wsab_sb[s][:], wsab_d[s][:])
                qs[(qi + 1) % 3].dma_start(xt_sb[:, s, :], xt_d[s])
                qi += 2
            # tiny [128,1] bias load (128 4-byte descriptors) goes last so
            # it never blocks the weight/activation streams
            nc.scalar.dma_start(bn_sb[:], bn_d[:])

            dd = sb.tile([128, 2 * KB], f32, name="dd")
            es = {}

            def phase_c(g):
                out2 = o2p.tile([128, WMAX], f32, tag="o2", name="o2")
                for c in range(4):
                    for j in range(4):
                        s = 4 * g + j
                        W = Wlist[s]
                        w1 = W + 1
                        p, a = divmod(c, 2)
                        ev = es[(s, p)][:, a * w1 : a * w1 + W]
                        lhsT = y2[:, 2 * (4 * s + c) : 2 * (4 * s + c) + 2]
                        for lo, hi in _pieces(0, W):
                            nc.tensor.matmul(
                                out2[32 * j : 32 * j + 2, lo:hi],
                                lhsT,
                                ev[:, lo:hi],
                                start=(c == 0),
                                stop=(c == 3),
                                tile_position=(0, 32 * j),
                                skip_group_check=True,
                            )
                ts_g = sb.tile([128, WMAX], bf16, tag="ts", name="ts")
                half = (WMAX // 2 + 16) // 32 * 32
                nc.scalar.copy(ts_g[:, 0:half], out2[:, 0:half])
                nc.vector.tensor_copy(ts_g[:, half:WMAX], out2[:, half:WMAX])
                T_g = sb.tile([128, WMAX], bf16, tag="T", name="T")
                Tv = T_g.rearrange("q (k b) -> q k b", b=32)
                rs_g = sb.tile([128, KB], f32, tag=f"rs{g}", name=f"rs{g}")
                r_g = sb.tile([128, KB], f32, tag=f"r{g}", name=f"r{g}")
                u_g = sb.tile([128, KB], f32, tag=f"u{g}", name=f"u{g}")
                up_g = sb.tile([128, KB], f32, tag=f"up{g}", name=f"up{g}")
                # d = 1 / (1 + exp(-(r + bs))); the trailing group runs in
                # two halves so the transpose->...->recip chain pipelines
                splits = [(0, half), (half, WMAX)] if g == 1 else [(0, WMAX)]
                for lo, hi in splits:
                    kl, kh = lo // 32, hi // 32
                    nc.vector.transpose(T_g[:, lo:hi], ts_g[:, lo:hi])
                    nc.vector.reciprocal(rs_g[:, kl:kh], Tv[:, kl:kh, 1])
                    nc.vector.tensor_mul(
                        r_g[:, kl:kh], Tv[:, kl:kh, 0], rs_g[:, kl:kh]
                    )
                    nc.scalar.activation(
                        u_g[:, kl:kh], r_g[:, kl:kh], Exp, bias=bn_sb,
                        scale=-1.0,
                    )
                    nc.vector.tensor_scalar_add(
                        up_g[:, kl:kh], u_g[:, kl:kh], 1.0
                    )
                    nc.vector.reciprocal(
                        dd[:, g * KB + kl : g * KB + kh], up_g[:, kl:kh]
                    )

            for s in range(BLOC):
                W = Wlist[s]
                w1 = W + 1
                for p in range(2):
                    lg = lgp.tile([128, TW], f32, tag="lg", name="lg")
                    for a in range(2):
                        c = 2 * p + a
                        xt_c = xt_sb[:, s, 128 * c : 128 * (c + 1)]
                        base = a * w1
                        for lo, hi in _pieces(base, base + w1):
                            nc.tensor.matmul(
                                lg[:, lo:hi],
                                xt_c,
                                wsab_sb[s][:, lo - base : hi - base],
                            )
                    # y for chunks (2p, 2p+1) sits at lg cols W and 2W+1;
                    # extract first so the DVE finishes this tile sooner
                    kA = 4 * s + 2 * p
                    yv = lg[:, 0 : 2 * w1].rearrange(
                        "q (two w) -> q two w", two=2
                    )[:, :, W]
                    nc.vector.tensor_copy(y2v[:, kA : kA + 2, 0], yv)
                    etile = sb.tile(
                        [128, TW], bf16, tag=f"e{s}_{p}", name=f"e{s}_{p}"
                    )
                    # split at the chunk boundary: PE can refill the first
                    # chunk's PSUM region as soon as ACT alone has read it
                    S = (w1 + 1) // 2 * 2
                    nc.scalar.activation(
                        etile[:, 0:S], lg[:, 0:S], Exp, scale=LN2
                    )
                    nc.vector.tensor_scalar(
                        etile.bitcast(i16)[:, S : 2 * w1],
                        lg[:, S : 2 * w1],
                        128.0,
                        SCHRAUD_B,
                        MULT,
                        ADD,
                    )
                    es[(s, p)] = etile
                if s == 5:
                    phase_c(0)
            phase_c(1)

            nc.sync.dma_start(
                out_d[:], dd.rearrange("q (g k) -> q g k", g=2)
            )

    nc.compile()
    return nc


def _prep(X, K, Wa, Ws, bs):
    import ml_dtypes

    bf16 = ml_dtypes.bfloat16
    X = np.asarray(X, dtype=np.float32)
    kpos = np.asarray(K) > 0
    Wa = np.asarray(Wa, dtype=np.float32)
    Ws = np.asarray(Ws, dtype=np.float32)
    bsv = float(np.asarray(bs, dtype=np.float32).reshape(-1)[0])

    nb = kpos.sum(1)
    order = np.argsort(-nb, kind="stable")  # rank r -> batch
    Wlist = []
    for s in range(BLOC):
        w = int(nb[order[8 * s : 8 * s + 8]].max())
        w = (w + 31) // 32 * 32 if s == 0 else (w + 7) // 8 * 8
        Wlist.append(w)
    KB = Wlist[0] // 32

    Wa2 = Wa * LOG2E
    XT = np.ascontiguousarray(np.transpose(X, (0, 2, 1))).astype(bf16)
    bn = np.full((128, 1), -bsv, dtype=np.float32)

    in_maps = []
    idxs = {}
    for c in range(NCORES):
        m = {"bn": bn}
        bats = [order[8 * s + c] for s in range(BLOC)]
        m["xt"] = np.ascontiguousarray(XT[bats])
        for s in range(BLOC):
            b = bats[s]
            idx = np.flatnonzero(kpos[b])
            idxs[(c, s)] = (b, idx)
            wsab = np.zeros((L, Wlist[s] + 1), dtype=np.float32)
            wsab[:, : len(idx)] = Wa2[:, idx]
            wsab[:, Wlist[s]] = Ws
            m[f"wsab{s}"] = wsab.astype(bf16)
        in_maps.append(m)
    return in_maps, idxs, Wlist, KB


def _run(X, K, Wa, Ws, bs, **spmd_kwargs):
    from concourse.bass_utils import run_bass_kernel_spmd

    in_maps, idxs, Wlist, KB = _prep(X, K, Wa, Ws, bs)
    key = tuple(Wlist)
    if _STATE.get("key") != key:
        _STATE["nc"] = _build(Wlist, KB)
        _STATE["key"] = key
    nc = _STATE["nc"]

    res = run_bass_kernel_spmd(
        nc, in_maps, core_ids=list(range(NCORES)), **spmd_kwargs
    )
    out = np.zeros((B, M), dtype=np.float32)
    for c in range(NCORES):
        o = res.results[c]["out"]  # (128, 2, KB)
        dp = (
            o.reshape(4, 32, 2, KB)
            .transpose(2, 0, 3, 1)
            .reshape(BLOC, KB * 32)
        )
        for s in range(BLOC):
            b, idx = idxs[(c, s)]
            out[b, idx] = dp[s, : len(idx)]
    return out, res


def kernel(X, K, Wa, ba, Ws, bs):
    out, _ = _run(X, K, Wa, Ws, bs)
    return out


def kernel_traced(X, K, Wa, ba, Ws, bs):
    out, res = _run(X, K, Wa, Ws, bs, trace=False)
    return out, res


# revision 11
# speedup vs baseline: 1.3208x; 1.3208x over previous
"""Trainium2 Bass kernel for nn_KnowledgeDifficulty (ragged active-column version).

Math (per batch b):
  logits = X[b] @ Wa + ba            (N, M)
  w      = softmax(logits, axis=N)   -- ba constant along N => cancels
  d      = sigmoid((sum_n e[n,m] * y[n]) / (sum_n e[n,m]) + bs)
    where e = exp(logits), y = X[b] @ Ws
  out    = d * (K > 0)

Ragged trick: only the ~50% columns with K>0 are needed.  The host gathers
the active columns of Wa per batch (scaled by log2(e): device works in
exp2 units), pads each batch to a fixed per-slot width, and scatters the
packed device output back to the full (B, M) array.

Device (per core, 8 batches = 8 "slots"):
  mm1 (bf16): logits2[n, c] = xt_chunk^T @ wsab (wsab = [Wa_act*log2e | Ws]),
       column W of each chunk region holds y for that chunk.
  exp: each PSUM tile's columns are split between ACT (spline exp,
       scale=ln2) and DVE (Schraudolph: i16 = round(z*128 + 16250.49)
       bitcast to bf16 ~= 2^z, +-3% rel) so both engines run on every
       tile concurrently.
  mm2 (bf16): per group of 4 slots, PE column-group packing: lhsT=[y|1]
       gives rows t=sum(e*y), s=sum(e) at partitions 32j/32j+1.
  epilogue: DVE 32x32 block-transpose puts t/s into columns; vectorized
       recip/mul/exp/add/recip gives d; one small DMA out, host unshuffles.

Batches are assigned to (core, slot) sorted by active count so each slot's
static width is tight; widths are baked into the NEFF at first call.
"""

import numpy as np

B, N, L, M = 64, 512, 128, 1024
NCORES = 8
BLOC = B // NCORES  # 8 slots per core
LOG2E = 1.4426950408889634
LN2 = 0.6931471805599453
SCHRAUD_B = 16256.0 - 5.51  # bf16 exp2 bitcast bias (round-to-nearest)
ACT_FRAC = 0.55  # fraction of each tile's exp columns done by ACT

_STATE = {}


def _pieces(lo, hi):
    # split [lo, hi) at multiples of 512 (PSUM bank = 512 fp32)
    out = []
    while lo < hi:
        nxt = min(hi, (lo // 512 + 1) * 512)
        out.append((lo, nxt))
        lo = nxt
    return out


def _build(Wlist, KB):
    import concourse.bacc as bacc
    import concourse.tile as tile
    import concourse.mybir as mybir

    f32 = mybir.dt.float32
    bf16 = mybir.dt.bfloat16
    i16 = mybir.dt.int16
    Exp = mybir.ActivationFunctionType.Exp
    MULT = mybir.AluOpType.mult
    ADD = mybir.AluOpType.add

    WMAX = Wlist[0]
    TW = 2 * WMAX + 2  # lg tile width (2 chunks + 2 y cols)

    nc = bacc.Bacc(
        "TRN2", target_bir_lowering=False, debug=False, num_devices=NCORES
    )
    wsab_d = [
        nc.dram_tensor(f"wsab{s}", (L, Wlist[s] + 1), bf16, kind="ExternalInput")
        for s in range(BLOC)
    ]
    xt_d = nc.dram_tensor("xt", (BLOC, L, N), bf16, kind="ExternalInput")
    bn_d = nc.dram_tensor("bn", (128, 1), f32, kind="ExternalInput")
    out_d = nc.dram_tensor("out", (128, 2, KB), f32, kind="ExternalOutput")

    with tile.TileContext(nc) as tc:
        with (
            tc.tile_pool(name="sb", bufs=1) as sb,
            tc.tile_pool(name="lgp", bufs=2, space="PSUM") as lgp,
            tc.tile_pool(name="o2p", bufs=1, space="PSUM") as o2p,
        ):
            bn_sb = sb.tile([128, 1], f32, name="bn_sb")
            # y2 interleaved: col 2k = y for (slot,chunk) k, col 2k+1 = 1.0
            y2 = sb.tile([128, 64], bf16, name="y2")
            y2v = y2.rearrange("q (k two) -> q k two", two=2)
            nc.vector.memset(y2v[:, :, 1], 1.0)

            xt_sb = sb.tile([L, BLOC, N], bf16, name="xt_sb")
            wsab_sb = []
            # slot 0 split finely over sync+gpsimd (scalar queue does the
            # ACT table load first); later slots rotate over all three
            for s in range(BLOC):
                w_sb = sb.tile(
                    [L, Wlist[s] + 1], bf16, tag=f"w{s}", name=f"w{s}"
                )
                wsab_sb.append(w_sb)
            qs = [nc.sync, nc.gpsimd, nc.scalar]
            for h in range(4):
                q = (nc.sync, nc.gpsimd)[h % 2]
                q.dma_start(
                    wsab_sb[0][32 * h : 32 * h + 32, :],
                    wsab_d[0][32 * h : 32 * h + 32, :],
                )
            nc.sync.dma_start(xt_sb[:, 0, 0:256], xt_d[0][:, 0:256])
            nc.gpsimd.dma_start(xt_sb[:, 0, 256:512], xt_d[0][:, 256:512])
            qi = 2
            for s in range(1, BLOC):
                qs[qi % 3].dma_start(wsab_sb[s][:], wsab_d[s][:])
                qs[(qi + 1) % 3].dma_start(xt_sb[:, s, :], xt_d[s])
                qi += 2
            # tiny [128,1] bias load (128 4-byte descriptors) goes last so
            # it never blocks the weight/activation streams
            nc.scalar.dma_start(bn_sb[:], bn_d[:])

            dd = sb.tile([128, 2 * KB], f32, name="dd")
            es = {}

            def phase_c(g):
                out2 = o2p.tile([128, WMAX], f32, tag="o2", name="o2")
                for c in range(4):
                    for j in range(4):
                        s = 4 * g + j
                        W = Wlist[s]
                        w1 = W + 1
                        p, a = divmod(c, 2)
                        ev = es[(s, p)][:, a * w1 : a * w1 + W]
                        lhsT = y2[:, 2 * (4 * s + c) : 2 * (4 * s + c) + 2]
                        for lo, hi in _pieces(0, W):
                            nc.tensor.matmul(
                                out2[32 * j : 32 * j + 2, lo:hi],
                                lhsT,
                                ev[:, lo:hi],
                                start=(c == 0),
                                stop=(c == 3),
                                tile_position=(0, 32 * j),
                                skip_group_check=True,
                            )
                ts_g = sb.tile([128, WMAX], bf16, tag="ts", name="ts")
                half = (WMAX // 2 + 16) // 32 * 32
                nc.scalar.copy(ts_g[:, 0:half], out2[:, 0:half])
                nc.vector.tensor_copy(ts_g[:, half:WMAX], out2[:, half:WMAX])
                T_g = sb.tile([128, WMAX], bf16, tag="T", name="T")
                Tv = T_g.rearrange("q (k b) -> q k b", b=32)
                rs_g = sb.tile([128, KB], f32, tag=f"rs{g}", name=f"rs{g}")
                r_g = sb.tile([128, KB], f32, tag=f"r{g}", name=f"r{g}")
                u_g = sb.tile([128, KB], f32, tag=f"u{g}", name=f"u{g}")
                up_g = sb.tile([128, KB], f32, tag=f"up{g}", name=f"up{g}")
                # d = 1 / (1 + exp(-(r + bs))); the trailing group runs in
                # two halves so the transpose->...->recip chain pipelines
                splits = [(0, half), (half, WMAX)] if g == 1 else [(0, WMAX)]
                for lo, hi in splits:
                    kl, kh = lo // 32, hi // 32
                    nc.vector.transpose(T_g[:, lo:hi], ts_g[:, lo:hi])
                    nc.vector.reciprocal(rs_g[:, kl:kh], Tv[:, kl:kh, 1])
                    nc.vector.tensor_mul(
                        r_g[:, kl:kh], Tv[:, kl:kh, 0], rs_g[:, kl:kh]
                    )
                    nc.scalar.activation(
                        u_g[:, kl:kh], r_g[:, kl:kh], Exp, bias=bn_sb,
                        scale=-1.0,
                    )
                    nc.vector.tensor_scalar_add(
                        up_g[:, kl:kh], u_g[:, kl:kh], 1.0
                    )
                    nc.vector.reciprocal(
                        dd[:, g * KB + kl : g * KB + kh], up_g[:, kl:kh]
                    )

            for s in range(BLOC):
                W = Wlist[s]
                w1 = W + 1
                for p in range(2):
                    lg = lgp.tile([128, TW], f32, tag="lg", name="lg")
                    for a in range(2):
                        c = 2 * p + a
                        xt_c = xt_sb[:, s, 128 * c : 128 * (c + 1)]
                        base = a * w1
                        for lo, hi in _pieces(base, base + w1):
                            nc.tensor.matmul(
                                lg[:, lo:hi],
                                xt_c,
                                wsab_sb[s][:, lo - base : hi - base],
                            )
                    # y for chunks (2p, 2p+1) sits at lg cols W and 2W+1;
                    # extract first so the DVE finishes this tile sooner
                    kA = 4 * s + 2 * p
                    yv = lg[:, 0 : 2 * w1].rearrange(
                        "q (two w) -> q two w", two=2
                    )[:, :, W]
                    nc.vector.tensor_copy(y2v[:, kA : kA + 2, 0], yv)
                    etile = sb.tile(
                        [128, TW], bf16, tag=f"e{s}_{p}", name=f"e{s}_{p}"
                    )
                    S = int(ACT_FRAC * 2 * w1) // 2 * 2
                    nc.scalar.activation(
                        etile[:, 0:S], lg[:, 0:S], Exp, scale=LN2
                    )
                    nc.vector.tensor_scalar(
                        etile.bitcast(i16)[:, S : 2 * w1],
                        lg[:, S : 2 * w1],
                        128.0,
                        SCHRAUD_B,
                        MULT,
                        ADD,
                    )
                    es[(s, p)] = etile
                if s == 5:
                    phase_c(0)
            phase_c(1)

            nc.sync.dma_start(
                out_d[:], dd.rearrange("q (g k) -> q g k", g=2)
            )

    nc.compile()
    return nc


def _prep(X, K, Wa, Ws, bs):
    import ml_dtypes

    bf16 = ml_dtypes.bfloat16
    X = np.asarray(X, dtype=np.float32)
    kpos = np.asarray(K) > 0
    Wa = np.asarray(Wa, dtype=np.float32)
    Ws = np.asarray(Ws, dtype=np.float32)
    bsv = float(np.asarray(bs, dtype=np.float32).reshape(-1)[0])

    nb = kpos.sum(1)
    order = np.argsort(-nb, kind="stable")  # rank r -> batch
    Wlist = []
    for s in range(BLOC):
        w = int(nb[order[8 * s : 8 * s + 8]].max())
        w = (w + 31) // 32 * 32 if s == 0 else (w + 7) // 8 * 8
        Wlist.append(w)
    KB = Wlist[0] // 32

    Wa2 = Wa * LOG2E
    XT = np.ascontiguousarray(np.transpose(X, (0, 2, 1))).astype(bf16)
    bn = np.full((128, 1), -bsv, dtype=np.float32)

    in_maps = []
    idxs = {}
    for c in range(NCORES):
        m = {"bn": bn}
        bats = [order[8 * s + c] for s in range(BLOC)]
        m["xt"] = np.ascontiguousarray(XT[bats])
        for s in range(BLOC):
            b = bats[s]
            idx = np.flatnonzero(kpos[b])
            idxs[(c, s)] = (b, idx)
            wsab = np.zeros((L, Wlist[s] + 1), dtype=np.float32)
            wsab[:, : len(idx)] = Wa2[:, idx]
            wsab[:, Wlist[s]] = Ws
            m[f"wsab{s}"] = wsab.astype(bf16)
        in_maps.append(m)
    return in_maps, idxs, Wlist, KB


def _run(X, K, Wa, Ws, bs, **spmd_kwargs):
    from concourse.bass_utils import run_bass_kernel_spmd

    in_maps, idxs, Wlist, KB = _prep(X, K, Wa, Ws, bs)
    key = tuple(Wlist)
    if _STATE.get("key") != key:
        _STATE["nc"] = _build(Wlist, KB)
        _STATE["key"] = key
    nc = _STATE["nc"]

    res = run_bass_kernel_spmd(
        nc, in_maps, core_ids=list(range(NCORES)), **spmd_kwargs
    )
    out = np.zeros((B, M), dtype=np.float32)
    for c in range(NCORES):
        o = res.results[c]["out"]  # (128, 2, KB)
        dp = (
            o.reshape(4, 32, 2, KB)
            .transpose(2, 0, 3, 1)
            .reshape(BLOC, KB * 32)
        )
        for s in range(BLOC):
            b, idx = idxs[(c, s)]
            out[b, idx] = dp[s, : len(idx)]
    return out, res


def kernel(X, K, Wa, ba, Ws, bs):
    out, _ = _run(X, K, Wa, Ws, bs)
    return out


def kernel_traced(X, K, Wa, ba, Ws, bs):
    out, res = _run(X, K, Wa, Ws, bs, trace=False)
    return out, res


# revision 14
# speedup vs baseline: 1.3271x; 1.0048x over previous
"""Trainium2 Bass kernel for nn_KnowledgeDifficulty (ragged active-column version).

Math (per batch b):
  logits = X[b] @ Wa + ba            (N, M)
  w      = softmax(logits, axis=N)   -- ba constant along N => cancels
  d      = sigmoid((sum_n e[n,m] * y[n]) / (sum_n e[n,m]) + bs)
    where e = exp(logits), y = X[b] @ Ws
  out    = d * (K > 0)

Ragged trick: only the ~50% columns with K>0 are needed.  The host gathers
the active columns of Wa per batch (scaled by log2(e): device works in
exp2 units), pads each batch to a fixed per-slot width, and scatters the
packed device output back to the full (B, M) array.

Device (per core, 8 batches = 8 "slots"):
  mm1 (bf16): logits2[n, c] = xt_chunk^T @ wsab (wsab = [Wa_act*log2e | Ws]),
       column W of each chunk region holds y for that chunk.
  exp: each PSUM tile's columns are split between ACT (spline exp,
       scale=ln2) and DVE (Schraudolph: i16 = round(z*128 + 16250.49)
       bitcast to bf16 ~= 2^z, +-3% rel) so both engines run on every
       tile concurrently.
  mm2 (bf16): per group of 4 slots, PE column-group packing: lhsT=[y|1]
       gives rows t=sum(e*y), s=sum(e) at partitions 32j/32j+1.
  epilogue: DVE 32x32 block-transpose puts t/s into columns; vectorized
       recip/mul/exp/add/recip gives d; one small DMA out, host unshuffles.

Batches are assigned to (core, slot) sorted by active count so each slot's
static width is tight; widths are baked into the NEFF at first call.
"""

import numpy as np

B, N, L, M = 64, 512, 128, 1024
NCORES = 8
BLOC = B // NCORES  # 8 slots per core
LOG2E = 1.4426950408889634
LN2 = 0.6931471805599453
SCHRAUD_B = 16256.0 - 5.51  # bf16 exp2 bitcast bias (round-to-nearest)
ACT_FRAC = 0.55  # fraction of each tile's exp columns done by ACT

_STATE = {}


def _pieces(lo, hi):
    # split [lo, hi) at multiples of 512 (PSUM bank = 512 fp32)
    out = []
    while lo < hi:
        nxt = min(hi, (lo // 512 + 1) * 512)
        out.append((lo, nxt))
        lo = nxt
    return out


def _build(Wlist, KB):
    import concourse.bacc as bacc
    import concourse.tile as tile
    import concourse.mybir as mybir

    f32 = mybir.dt.float32
    bf16 = mybir.dt.bfloat16
    i16 = mybir.dt.int16
    Exp = mybir.ActivationFunctionType.Exp
    MULT = mybir.AluOpType.mult
    ADD = mybir.AluOpType.add

    WMAX = Wlist[0]
    TW = 2 * WMAX + 2  # lg tile width (2 chunks + 2 y cols)

    nc = bacc.Bacc(
        "TRN2", target_bir_lowering=False, debug=False, num_devices=NCORES
    )
    wsab_d = [
        nc.dram_tensor(f"wsab{s}", (L, Wlist[s] + 1), bf16, kind="ExternalInput")
        for s in range(BLOC)
    ]
    xt_d = nc.dram_tensor("xt", (BLOC, L, N), bf16, kind="ExternalInput")
    bn_d = nc.dram_tensor("bn", (128, 1), f32, kind="ExternalInput")
    out_d = nc.dram_tensor("out", (128, 2, KB), f32, kind="ExternalOutput")

    with tile.TileContext(nc) as tc:
        with (
            tc.tile_pool(name="sb", bufs=1) as sb,
            tc.tile_pool(name="lgp", bufs=2, space="PSUM") as lgp,
            tc.tile_pool(name="o2p", bufs=1, space="PSUM") as o2p,
        ):
            bn_sb = sb.tile([128, 1], f32, name="bn_sb")
            # y2 interleaved: col 2k = y for (slot,chunk) k, col 2k+1 = 1.0
            y2 = sb.tile([128, 64], bf16, name="y2")
            y2v = y2.rearrange("q (k two) -> q k two", two=2)
            nc.vector.memset(y2v[:, :, 1], 1.0)

            xt_sb = sb.tile([L, BLOC, N], bf16, name="xt_sb")
            wsab_sb = []
            # slot 0 split finely over sync+gpsimd (scalar queue does the
            # ACT table load first); later slots rotate over all three
            for s in range(BLOC):
                w_sb = sb.tile(
                    [L, Wlist[s] + 1], bf16, tag=f"w{s}", name=f"w{s}"
                )
                wsab_sb.append(w_sb)
            qs = [nc.sync, nc.gpsimd, nc.scalar]
            for h in range(4):
                q = (nc.sync, nc.gpsimd)[h % 2]
                q.dma_start(
                    wsab_sb[0][32 * h : 32 * h + 32, :],
                    wsab_d[0][32 * h : 32 * h + 32, :],
                )
            nc.sync.dma_start(xt_sb[:, 0, 0:256], xt_d[0][:, 0:256])
            nc.gpsimd.dma_start(xt_sb[:, 0, 256:512], xt_d[0][:, 256:512])
            qi = 2
            for s in range(1, BLOC):
                qs[qi % 3].dma_start(wsab_sb[s][:], wsab_d[s][:])
                qs[(qi + 1) % 3].dma_start(xt_sb[:, s, :], xt_d[s])
                qi += 2
            # tiny [128,1] bias load (128 4-byte descriptors) goes last so
            # it never blocks the weight/activation streams
            nc.scalar.dma_start(bn_sb[:], bn_d[:])

            dd = sb.tile([128, 2 * KB], f32, name="dd")
            # one 2-bank PSUM tile: cols [0,WMAX) = mm2 accumulator (both
            # groups, WAR-ordered), cols [576,608) = per-(slot,chunk) y
            o2t = o2p.tile([128, 1024], f32, name="o2t")
            out2 = o2t[:, 0:WMAX]
            yps = o2t[:, 576:608]
            es = {}

            def phase_c(g):
                for c in range(4):
                    for j in range(4):
                        s = 4 * g + j
                        W = Wlist[s]
                        w1 = W + 1
                        p, a = divmod(c, 2)
                        ev = es[(s, p)][:, a * w1 : a * w1 + W]
                        lhsT = y2[:, 2 * (4 * s + c) : 2 * (4 * s + c) + 2]
                        for lo, hi in _pieces(0, W):
                            nc.tensor.matmul(
                                out2[32 * j : 32 * j + 2, lo:hi],
                                lhsT,
                                ev[:, lo:hi],
                                start=(c == 0),
                                stop=(c == 3),
                                tile_position=(0, 32 * j),
                                skip_group_check=True,
                            )
                ts_g = sb.tile([128, WMAX], bf16, tag="ts", name="ts")
                half = (WMAX // 2 + 16) // 32 * 32
                nc.scalar.copy(ts_g[:, 0:half], out2[:, 0:half])
                nc.vector.tensor_copy(ts_g[:, half:WMAX], out2[:, half:WMAX])
                T_g = sb.tile([128, WMAX], bf16, tag="T", name="T")
                Tv = T_g.rearrange("q (k b) -> q k b", b=32)
                rs_g = sb.tile([128, KB], f32, tag=f"rs{g}", name=f"rs{g}")
                r_g = sb.tile([128, KB], f32, tag=f"r{g}", name=f"r{g}")
                u_g = sb.tile([128, KB], f32, tag=f"u{g}", name=f"u{g}")
                up_g = sb.tile([128, KB], f32, tag=f"up{g}", name=f"up{g}")
                # d = 1 / (1 + exp(-(r + bs))); the trailing group runs in
                # two halves so the transpose->...->recip chain pipelines
                splits = [(0, half), (half, WMAX)] if g == 1 else [(0, WMAX)]
                for lo, hi in splits:
                    kl, kh = lo // 32, hi // 32
                    nc.vector.transpose(T_g[:, lo:hi], ts_g[:, lo:hi])
                    nc.vector.reciprocal(rs_g[:, kl:kh], Tv[:, kl:kh, 1])
                    nc.vector.tensor_mul(
                        r_g[:, kl:kh], Tv[:, kl:kh, 0], rs_g[:, kl:kh]
                    )
                    nc.scalar.activation(
                        u_g[:, kl:kh], r_g[:, kl:kh], Exp, bias=bn_sb,
                        scale=-1.0,
                    )
                    nc.vector.tensor_scalar_add(
                        up_g[:, kl:kh], u_g[:, kl:kh], 1.0
                    )
                    nc.vector.reciprocal(
                        dd[:, g * KB + kl : g * KB + kh], up_g[:, kl:kh]
                    )

            for s in range(BLOC):
                W = Wlist[s]
                w1 = W + 1
                for p in range(2):
                    lg = lgp.tile([128, TW], f32, tag="lg", name="lg")
                    for a in range(2):
                        c = 2 * p + a
                        xt_c = xt_sb[:, s, 128 * c : 128 * (c + 1)]
                        base = a * w1
                        for lo, hi in _pieces(base, base + w1):
                            nc.tensor.matmul(
                                lg[:, lo:hi],
                                xt_c,
                                wsab_sb[s][:, lo - base : hi - base],
                            )
                        # y for this chunk into the spare PSUM region so
                        # the exp engines alone gate the lg buffer recycle
                        k = 4 * s + c
                        nc.tensor.matmul(
                            yps[:, k : k + 1],
                            xt_c,
                            wsab_sb[s][:, W : W + 1],
                        )
                    etile = sb.tile(
                        [128, TW], bf16, tag=f"e{s}_{p}", name=f"e{s}_{p}"
                    )
                    S = int(ACT_FRAC * 2 * w1) // 2 * 2
                    nc.scalar.activation(
                        etile[:, 0:S], lg[:, 0:S], Exp, scale=LN2
                    )
                    nc.vector.tensor_scalar(
                        etile.bitcast(i16)[:, S : 2 * w1],
                        lg[:, S : 2 * w1],
                        128.0,
                        SCHRAUD_B,
                        MULT,
                        ADD,
                    )
                    es[(s, p)] = etile
                if s == 3:
                    nc.vector.tensor_copy(y2v[:, 0:16, 0], yps[:, 0:16])
                if s == 5:
                    phase_c(0)
            nc.vector.tensor_copy(y2v[:, 16:32, 0], yps[:, 16:32])
            phase_c(1)

            nc.sync.dma_start(
                out_d[:], dd.rearrange("q (g k) -> q g k", g=2)
            )

    nc.compile()
    return nc


def _prep(X, K, Wa, Ws, bs):
    import ml_dtypes

    bf16 = ml_dtypes.bfloat16
    X = np.asarray(X, dtype=np.float32)
    kpos = np.asarray(K) > 0
    Wa = np.asarray(Wa, dtype=np.float32)
    Ws = np.asarray(Ws, dtype=np.float32)
    bsv = float(np.asarray(bs, dtype=np.float32).reshape(-1)[0])

    nb = kpos.sum(1)
    order = np.argsort(-nb, kind="stable")  # rank r -> batch
    Wlist = []
    for s in range(BLOC):
        w = int(nb[order[8 * s : 8 * s + 8]].max())
        w = (w + 31) // 32 * 32 if s == 0 else (w + 7) // 8 * 8
        Wlist.append(w)
    KB = Wlist[0] // 32

    Wa2 = Wa * LOG2E
    XT = np.ascontiguousarray(np.transpose(X, (0, 2, 1))).astype(bf16)
    bn = np.full((128, 1), -bsv, dtype=np.float32)

    in_maps = []
    idxs = {}
    for c in range(NCORES):
        m = {"bn": bn}
        bats = [order[8 * s + c] for s in range(BLOC)]
        m["xt"] = np.ascontiguousarray(XT[bats])
        for s in range(BLOC):
            b = bats[s]
            idx = np.flatnonzero(kpos[b])
            idxs[(c, s)] = (b, idx)
            wsab = np.zeros((L, Wlist[s] + 1), dtype=np.float32)
            wsab[:, : len(idx)] = Wa2[:, idx]
            wsab[:, Wlist[s]] = Ws
            m[f"wsab{s}"] = wsab.astype(bf16)
        in_maps.append(m)
    return in_maps, idxs, Wlist, KB


def _run(X, K, Wa, Ws, bs, **spmd_kwargs):
    from concourse.bass_utils import run_bass_kernel_spmd

    in_maps, idxs, Wlist, KB = _prep(X, K, Wa, Ws, bs)
    key = tuple(Wlist)
    if _STATE.get("key") != key:
        _STATE["nc"] = _build(Wlist, KB)
        _STATE["key"] = key
    nc = _STATE["nc"]

    res = run_bass_kernel_spmd(
        nc, in_maps, core_ids=list(range(NCORES)), **spmd_kwargs
    )
    out = np.zeros((B, M), dtype=np.float32)
    for c in range(NCORES):
        o = res.results[c]["out"]  # (128, 2, KB)
        dp = (
            o.reshape(4, 32, 2, KB)
            .transpose(2, 0, 3, 1)
            .reshape(BLOC, KB * 32)
        )
        for s in range(BLOC):
            b, idx = idxs[(c, s)]
            out[b, idx] = dp[s, : len(idx)]
    return out, res


def kernel(X, K, Wa, ba, Ws, bs):
    out, _ = _run(X, K, Wa, Ws, bs)
    return out


def kernel_traced(X, K, Wa, ba, Ws, bs):
    out, res = _run(X, K, Wa, Ws, bs, trace=False)
    return out, res
